# revision 14
# baseline (speedup 1.0000x reference)
"""Dual-stream attention kernel for TRN2 — one batch element per core (v3).

Per-core computation (batch element b):
  qb^T = Wq @ q_b^T          [C, N]   fp16, transposed layout (c on partitions)
  kb^T = Wk @ k_b^T          [C, N]   fp16
  vcomb[tb][tok, h, 0:64]   = (v_b @ Wv^T)    per-head slices   (natural layout)
  vcomb[tb][tok, h, 64:128] = (v_img_b @ Wvim^T)
  per head pair ct (2 heads = one 128-partition q/k tile):
    S^T pair = kh @ qh^T      K=64 matmuls, row-paired (lower/upper head) into
                              one 2-bank PSUM tile
    E = exp(S^T * scale)      ONE ACT instr per kb over both heads [128,1024]
    U = [vh | vih]^T @ E      M=128: rows 0:64 x-stream, 64:128 img-stream
    r = ones^T @ E            M=1 matmuls, col-paired (strip 0 / strip 32),
                              both accumulated in one PSUM bank
    O = U * (1/r)             reciprocal read straight from PSUM; normalize
                              multiplies read U from PSUM (no drain copies);
                              DMA partition-shifts for the two misaligned halves
  x    = merge(O_x)  @ Wp^T  + bp     bias folded in as a K=1 ones-row matmul;
  x_im = merge(O_im) @ Wpi^T + bpi    outputs DMA'd DRAM-ward from PSUM

Engine placement: PE matmuls; ACT exp + stage-1 q/k PSUM drains; DVE vcomb
drains, reciprocals, normalize multiplies. GPSIMD has no PSUM port so it
stays idle. All matmul operands fp16; PSUM accumulation fp32.

build_module(loop_n=N) wraps the body in a hardware For_i loop for wall-clock
timing (amortizes the ~60 ms axon dispatch overhead); timing is
data-independent.
"""

import numpy as np
import concourse.bass as bass
import concourse.tile as tile
from concourse import bacc, mybir

P = 128
NTOK = 1024
C = 768
H = 12
DH = 64
CT = C // P  # 6 c-tiles
TB = NTOK // P  # 8 token blocks
QH = 2  # qt halves
KB = 8  # kt blocks
NQ = 512
SCALE = DH**-0.5
F32 = mybir.dt.float32
F16 = mybir.dt.float16
EXP = mybir.ActivationFunctionType.Exp
MULT = mybir.AluOpType.mult
ADD = mybir.AluOpType.add

XNAMES = ("xq", "xk", "xv", "xvi")
WNAMES = ("wq", "wk", "wv", "wvi", "wp", "wpi")


def build_module(num_devices=8, loop_n=1, stages="123"):
    nc = bacc.Bacc(
        "TRN2", target_bir_lowering=False, debug=False, num_devices=num_devices
    )
    d = {}
    for nm in XNAMES:
        d[nm] = nc.dram_tensor(nm, [C, NTOK], F16, kind="ExternalInput").ap()
    for nm in WNAMES:
        d[nm] = nc.dram_tensor(nm, [C, C], F16, kind="ExternalInput").ap()
    d["ones"] = nc.dram_tensor("ones", [P, P], F16, kind="ExternalInput").ap()
    d["bp"] = nc.dram_tensor("bp", [1, C], F16, kind="ExternalInput").ap()
    d["bpi"] = nc.dram_tensor("bpi", [1, C], F16, kind="ExternalInput").ap()
    xo = nc.dram_tensor("xo", [NTOK, C], F32, kind="ExternalOutput").ap()
    xio = nc.dram_tensor("xio", [NTOK, C], F32, kind="ExternalOutput").ap()

    with tile.TileContext(nc) as tc:
        with (
            tc.tile_pool(name="persist", bufs=1) as pp,
            tc.tile_pool(name="wstage", bufs=2) as wpool,
            tc.tile_pool(name="xstage", bufs=2) as xpool,
            tc.tile_pool(name="wk", bufs=8) as wk,
            tc.tile_pool(name="nrm", bufs=8) as nrm,
            tc.tile_pool(name="tmp", bufs=8) as tmpp,
            tc.tile_pool(name="ps", bufs=1, space="PSUM") as psp,
        ):
            qbt = pp.tile([P, CT, NTOK], F16, tag="qbt")
            kbt = pp.tile([P, CT, NTOK], F16, tag="kbt")
            # [v | vi] per head: lhsT for the combined AV matmul
            vcomb = pp.tile([P, TB, H, P], F16, tag="vcomb")
            axt = pp.tile([P, CT, NTOK], F16, tag="axt")
            ait = pp.tile([P, CT, NTOK], F16, tag="ait")
            onest = pp.tile([P, P], F16, tag="onest")
            bpr = pp.tile([1, C], F16, tag="bpr")
            bpir = pp.tile([1, C], F16, tag="bpir")

            # PSUM budget (8 banks): spair 2x2 + u 2 + r 1 + rp 1
            def spair_tile():
                return psp.tile(
                    [P, 2, NQ], F32, tag="spair", bufs=2, name="spair"
                )

            def stage1():
                nc.sync.dma_start(bpr[:], d["bp"])
                nc.sync.dma_start(bpir[:], d["bpi"])
                nc.sync.dma_start(onest[:], d["ones"])

                for src, wsrc, mode in (
                    ("xv", "wv", "nat_v"),
                    ("xvi", "wvi", "nat_vi"),
                    ("xq", "wq", "tr_q"),
                    ("xk", "wk", "tr_k"),
                ):
                    xt = xpool.tile([P, CT, NTOK], F16, tag="xt")
                    nc.sync.dma_start(
                        xt[:], d[src].rearrange("(ct p) n -> p ct n", p=P)
                    )
                    wt = wpool.tile([P, CT, C], F16, tag="wt")
                    nc.sync.dma_start(
                        wt[:], d[wsrc].rearrange("(ct p) c -> p ct c", p=P)
                    )
                    if mode.startswith("tr"):
                        dst = qbt if mode == "tr_q" else kbt
                        for co in range(CT):
                            ps = spair_tile()
                            for nh in range(QH):
                                for ci in range(CT):
                                    nc.tensor.matmul(
                                        ps[:, nh, :],
                                        wt[:, ci, co * P : (co + 1) * P],
                                        xt[:, ci, nh * NQ : (nh + 1) * NQ],
                                        start=(ci == 0),
                                        stop=(ci == CT - 1),
                                    )
                            nc.scalar.copy(dst[:, co, :], ps[:, 0:2, :])
                    else:
                        off = 0 if mode == "nat_v" else DH
                        for tb in range(TB):
                            ps = spair_tile()
                            for slot, c0, cw in ((0, 0, 512), (1, 512, 256)):
                                for ci in range(CT):
                                    nc.tensor.matmul(
                                        ps[:, slot, :cw],
                                        xt[:, ci, tb * P : (tb + 1) * P],
                                        wt[:, ci, c0 : c0 + cw],
                                        start=(ci == 0),
                                        stop=(ci == CT - 1),
                                    )
                            nc.vector.tensor_copy(
                                vcomb[:, tb, 0:8, off : off + DH],
                                ps[:, 0, :].rearrange("p (h dh) -> p h dh", dh=DH),
                            )
                            nc.vector.tensor_copy(
                                vcomb[:, tb, 8:12, off : off + DH],
                                ps[:, 1, 0:256].rearrange(
                                    "p (h dh) -> p h dh", dh=DH
                                ),
                            )

            def normalize(item):
                # all-SBUF fp16 tensor_tensors (2x DVE mode); partition-shift
                # DMAs fix up the two misaligned halves
                ct, qsl, ub_lo, ub_up, rpb_l, rpb_u = item
                nc.vector.tensor_tensor(
                    axt[0:DH, ct, qsl], ub_lo[0:DH, :], rpb_l[0:DH, :], MULT
                )
                t_il = tmpp.tile([P, NQ], F16, tag="tshift", bufs=4)
                nc.vector.tensor_tensor(
                    t_il[DH:P, :], ub_lo[DH:P, :], rpb_l[DH:P, :], MULT
                )
                nc.sync.dma_start(ait[0:DH, ct, qsl], t_il[DH:P, :])
                t_xu = tmpp.tile([P, NQ], F16, tag="tshift", bufs=4)
                nc.vector.tensor_tensor(
                    t_xu[0:DH, :], ub_up[0:DH, :], rpb_u[0:DH, :], MULT
                )
                nc.sync.dma_start(axt[DH:P, ct, qsl], t_xu[0:DH, :])
                nc.vector.tensor_tensor(
                    ait[DH:P, ct, qsl], ub_up[DH:P, :], rpb_u[DH:P, :], MULT
                )

            def stage2():
                # Cross-group software pipeline: while group g's scores/exps
                # stream, the PE's U/rowsum matmuls consume group g-1's exps
                # (finished ~8 kb earlier). The PE never waits on ACT, so its
                # matmul stream stays back-to-back and the HAM clock-gate
                # ramps to (and stays at) full rate.
                groups = [(ct, qh) for ct in range(CT) for qh in range(QH)]
                stash = []
                prev = None

                def consume(pv, j):
                    u_lo, u_up, r, es, h_lo, h_up = pv
                    ep = es[j]
                    st, sp = (j == 0), (j == KB - 1)
                    nc.tensor.matmul(
                        u_lo[:], vcomb[:, j, h_lo, :], ep[:, 0, :],
                        start=st, stop=sp,
                    )
                    nc.tensor.matmul(
                        u_up[:], vcomb[:, j, h_up, :], ep[:, 1, :],
                        start=st, stop=sp,
                    )
                    nc.tensor.matmul(
                        r[0:1, :], onest[:, 0:1], ep[:, 0, :],
                        start=st, stop=sp, skip_group_check=True,
                    )
                    nc.tensor.matmul(
                        r[32:33, :], onest[:, 0:1], ep[:, 1, :],
                        start=st, stop=sp, skip_group_check=True,
                    )

                def finish(pv, ct, qsl):
                    # drain U + rowsum recip + broadcast; normalize deferred
                    u_lo, u_up, r, es, h_lo, h_up = pv
                    ub_lo = wk.tile([P, NQ], F16, tag="ub", bufs=6)
                    ub_up = wk.tile([P, NQ], F16, tag="ub", bufs=6)
                    nc.vector.tensor_copy(ub_lo[:], u_lo[:])
                    nc.vector.tensor_copy(ub_up[:], u_up[:])
                    rc = nrm.tile([33, NQ], F16, tag="rc", bufs=4)
                    with nc.allow_low_precision(reason="softmax recip fp16"):
                        nc.vector.reciprocal(rc[0:1, :], r[0:1, :])
                        nc.vector.reciprocal(rc[32:33, :], r[32:33, :])
                    rc2 = nrm.tile([1, NQ], F16, tag="rc2", bufs=4)
                    nc.sync.dma_start(rc2[:], rc[32:33, :])
                    rpb_l = tmpp.tile([P, NQ], F16, tag="rpb", bufs=6)
                    rpb_u = tmpp.tile([P, NQ], F16, tag="rpb", bufs=6)
                    nc.gpsimd.partition_broadcast(rpb_l[:], rc[0:1, :])
                    nc.gpsimd.partition_broadcast(rpb_u[:], rc2[0:1, :])
                    stash.append((ct, qsl, ub_lo, ub_up, rpb_l, rpb_u))

                for g, (ct, qh) in enumerate(groups):
                    h_lo, h_up = 2 * ct, 2 * ct + 1
                    qsl = slice(qh * NQ, (qh + 1) * NQ)
                    u_lo = psp.tile([P, NQ], F32, tag="u", bufs=3)
                    u_up = psp.tile([P, NQ], F32, tag="u", bufs=3)
                    r = psp.tile([33, NQ], F32, tag="r", bufs=1)
                    es = []
                    for kb in range(KB):
                        ksl = slice(kb * P, (kb + 1) * P)
                        sp2 = spair_tile()
                        nc.tensor.matmul(
                            sp2[:, 0, :], kbt[0:DH, ct, ksl],
                            qbt[0:DH, ct, qsl], start=True, stop=True,
                        )
                        nc.tensor.matmul(
                            sp2[:, 1, :], kbt[DH:P, ct, ksl],
                            qbt[DH:P, ct, qsl], start=True, stop=True,
                        )
                        ep = wk.tile([P, 2, NQ], F16, tag="e", bufs=16)
                        nc.scalar.activation(
                            ep[:, 0:2, :], sp2[:, 0:2, :], EXP, scale=SCALE
                        )
                        es.append(ep)
                        if prev is not None:
                            consume(prev[0], kb)
                        if kb == 2 and stash:
                            normalize(stash.pop())
                    if prev is not None:
                        finish(prev[0], prev[1], prev[2])
                    prev = ((u_lo, u_up, r, es, h_lo, h_up), ct, qsl)

                # epilogue: drain the last group's pipeline
                for kb in range(KB):
                    consume(prev[0], kb)
                finish(prev[0], prev[1], prev[2])
                while stash:
                    normalize(stash.pop(0))

            def stage3():
                for dst_dram, src, w_nm, bias_t in (
                    (xo, axt, "wp", bpr),
                    (xio, ait, "wpi", bpir),
                ):
                    wt = wpool.tile([P, CT, C], F16, tag="wt")
                    nc.sync.dma_start(
                        wt[:], d[w_nm].rearrange("(ct p) c -> p ct c", p=P)
                    )
                    for tb in range(TB):
                        ps = spair_tile()
                        for slot, c0, cw in ((0, 0, 512), (1, 512, 256)):
                            nc.tensor.matmul(
                                ps[:, slot, :cw],
                                onest[0:1, 0:P],
                                bias_t[0:1, c0 : c0 + cw],
                                start=True, stop=False,
                            )
                            for ci in range(CT):
                                nc.tensor.matmul(
                                    ps[:, slot, :cw],
                                    src[:, ci, tb * P : (tb + 1) * P],
                                    wt[:, ci, c0 : c0 + cw],
                                    start=False,
                                    stop=(ci == CT - 1),
                                )
                        ot = wk.tile([P, C], F32, tag="ot", bufs=3)
                        nc.scalar.copy(ot[:, 0:512], ps[:, 0, :])
                        nc.scalar.copy(ot[:, 512:768], ps[:, 1, 0:256])
                        nc.sync.dma_start(
                            dst_dram[tb * P : (tb + 1) * P, :], ot[:]
                        )

            def body():
                if "1" in stages:
                    stage1()
                if "2" in stages:
                    stage2()
                if "3" in stages:
                    stage3()

            if loop_n == 1:
                body()
            else:
                with tc.For_i(0, loop_n, 1):
                    body()

    nc.compile()
    return nc


def make_in_maps(q, k, v, v_img, Wq, Wk, Wv, Wvim, Wp, bp, Wpi, bpi, n_cores=8):
    """Host-side prep: per-core transposed fp16 activations + shared fp16 weights."""
    f = np.float32
    h = np.float16
    shared = {
        "wq": np.asarray(Wq, f).T.astype(h),
        "wk": np.asarray(Wk, f).T.astype(h),
        "wv": np.asarray(Wv, f).T.astype(h),
        "wvi": np.asarray(Wvim, f).T.astype(h),
        "wp": np.asarray(Wp, f).T.astype(h),
        "wpi": np.asarray(Wpi, f).T.astype(h),
        "ones": np.ones((P, P), h),
        "bp": np.asarray(bp, f).reshape(1, C).astype(h),
        "bpi": np.asarray(bpi, f).reshape(1, C).astype(h),
    }
    q = np.asarray(q, f)
    k = np.asarray(k, f)
    v = np.asarray(v, f)
    vi = np.asarray(v_img, f)
    in_maps = []
    for b in range(n_cores):
        in_maps.append(
            {
                "xq": np.ascontiguousarray(q[:, b, :].T).astype(h),
                "xk": np.ascontiguousarray(k[:, b, :].T).astype(h),
                "xv": np.ascontiguousarray(v[:, b, :].T).astype(h),
                "xvi": np.ascontiguousarray(vi[:, b, :].T).astype(h),
                **shared,
            }
        )
    return in_maps


# ---------------------------------------------------------------------------
# Harness entry point: full inputs in, full outputs out.
# Shards batch B=8 across the 8 NeuronCores (data parallel), no collectives.
# ---------------------------------------------------------------------------

_NC_CACHE = {}


def _get_module():
    if "nc" not in _NC_CACHE:
        _NC_CACHE["nc"] = build_module(num_devices=8)
    return _NC_CACHE["nc"]


def kernel(q, k, v, v_img, Wq, Wk, Wv, Wvim, Wp, bp, Wpi, bpi):
    from concourse.bass_utils import run_bass_kernel_spmd

    B = np.asarray(q).shape[1]
    nc = _get_module()
    in_maps = make_in_maps(q, k, v, v_img, Wq, Wk, Wv, Wvim, Wp, bp, Wpi, bpi,
                           n_cores=B)
    res = run_bass_kernel_spmd(nc, in_maps, core_ids=list(range(B)), trace=False)
    x = np.stack([res.results[b]["xo"] for b in range(B)])
    x_im = np.stack([res.results[b]["xio"] for b in range(B)])
    return (x, x_im)


# revision 15
# speedup vs baseline: 1.0710x; 1.0710x over previous
"""Dual-stream attention kernel for TRN2 — one batch element per core (v3).

Per-core computation (batch element b):
  qb^T = Wq @ q_b^T          [C, N]   fp16, transposed layout (c on partitions)
  kb^T = Wk @ k_b^T          [C, N]   fp16
  vcomb[tb][tok, h, 0:64]   = (v_b @ Wv^T)    per-head slices   (natural layout)
  vcomb[tb][tok, h, 64:128] = (v_img_b @ Wvim^T)
  per head pair ct (2 heads = one 128-partition q/k tile):
    S^T pair = kh @ qh^T      K=64 matmuls, row-paired (lower/upper head) into
                              one 2-bank PSUM tile
    E = exp(S^T * scale)      ONE ACT instr per kb over both heads [128,1024]
    U = [vh | vih]^T @ E      M=128: rows 0:64 x-stream, 64:128 img-stream
    r = ones^T @ E            M=1 matmuls, col-paired (strip 0 / strip 32),
                              both accumulated in one PSUM bank
    O = U * (1/r)             reciprocal read straight from PSUM; normalize
                              multiplies read U from PSUM (no drain copies);
                              DMA partition-shifts for the two misaligned halves
  x    = merge(O_x)  @ Wp^T  + bp     bias folded in as a K=1 ones-row matmul;
  x_im = merge(O_im) @ Wpi^T + bpi    outputs DMA'd DRAM-ward from PSUM

Engine placement: PE matmuls; ACT exp + stage-1 q/k PSUM drains; DVE vcomb
drains, reciprocals, normalize multiplies. GPSIMD has no PSUM port so it
stays idle. All matmul operands fp16; PSUM accumulation fp32.

build_module(loop_n=N) wraps the body in a hardware For_i loop for wall-clock
timing (amortizes the ~60 ms axon dispatch overhead); timing is
data-independent.
"""

import numpy as np
import concourse.bass as bass
import concourse.tile as tile
from concourse import bacc, mybir

P = 128
NTOK = 1024
C = 768
H = 12
DH = 64
CT = C // P  # 6 c-tiles
TB = NTOK // P  # 8 token blocks
QH = 2  # qt halves
KB = 8  # kt blocks
NQ = 512
SCALE = DH**-0.5
F32 = mybir.dt.float32
F16 = mybir.dt.float16
EXP = mybir.ActivationFunctionType.Exp
MULT = mybir.AluOpType.mult
ADD = mybir.AluOpType.add

XNAMES = ("xq", "xk", "xv", "xvi")
WNAMES = ("wq", "wk", "wv", "wvi", "wp", "wpi")


def build_module(num_devices=8, loop_n=1, stages="123"):
    nc = bacc.Bacc(
        "TRN2", target_bir_lowering=False, debug=False, num_devices=num_devices
    )
    d = {}
    for nm in XNAMES:
        d[nm] = nc.dram_tensor(nm, [C, NTOK], F16, kind="ExternalInput").ap()
    for nm in WNAMES:
        d[nm] = nc.dram_tensor(nm, [C, C], F16, kind="ExternalInput").ap()
    d["ones"] = nc.dram_tensor("ones", [P, P], F16, kind="ExternalInput").ap()
    d["bp"] = nc.dram_tensor("bp", [1, C], F16, kind="ExternalInput").ap()
    d["bpi"] = nc.dram_tensor("bpi", [1, C], F16, kind="ExternalInput").ap()
    xo = nc.dram_tensor("xo", [NTOK, C], F32, kind="ExternalOutput").ap()
    xio = nc.dram_tensor("xio", [NTOK, C], F32, kind="ExternalOutput").ap()

    with tile.TileContext(nc) as tc:
        with (
            tc.tile_pool(name="persist", bufs=1) as pp,
            tc.tile_pool(name="wstage", bufs=2) as wpool,
            tc.tile_pool(name="xstage", bufs=2) as xpool,
            tc.tile_pool(name="wk", bufs=8) as wk,
            tc.tile_pool(name="nrm", bufs=8) as nrm,
            tc.tile_pool(name="tmp", bufs=8) as tmpp,
            tc.tile_pool(name="ps", bufs=1, space="PSUM") as psp,
        ):
            qbt = pp.tile([P, CT, NTOK], F16, tag="qbt")
            kbt = pp.tile([P, CT, NTOK], F16, tag="kbt")
            # [v | vi] per head: lhsT for the combined AV matmul
            vcomb = pp.tile([P, TB, H, P], F16, tag="vcomb")
            axt = pp.tile([P, CT, NTOK], F16, tag="axt")
            ait = pp.tile([P, CT, NTOK], F16, tag="ait")
            onest = pp.tile([P, P], F16, tag="onest")
            bpr = pp.tile([1, C], F16, tag="bpr")
            bpir = pp.tile([1, C], F16, tag="bpir")

            # PSUM budget (8 banks): spair 2x2 + u 2 + r 1 + rp 1
            def spair_tile():
                return psp.tile(
                    [P, 2, NQ], F32, tag="spair", bufs=2, name="spair"
                )

            def stage1():
                nc.sync.dma_start(bpr[:], d["bp"])
                nc.sync.dma_start(bpir[:], d["bpi"])
                nc.sync.dma_start(onest[:], d["ones"])

                for src, wsrc, mode in (
                    ("xv", "wv", "nat_v"),
                    ("xvi", "wvi", "nat_vi"),
                    ("xq", "wq", "tr_q"),
                    ("xk", "wk", "tr_k"),
                ):
                    xt = xpool.tile([P, CT, NTOK], F16, tag="xt")
                    nc.sync.dma_start(
                        xt[:], d[src].rearrange("(ct p) n -> p ct n", p=P)
                    )
                    wt = wpool.tile([P, CT, C], F16, tag="wt")
                    nc.sync.dma_start(
                        wt[:], d[wsrc].rearrange("(ct p) c -> p ct c", p=P)
                    )
                    if mode.startswith("tr"):
                        dst = qbt if mode == "tr_q" else kbt
                        for co in range(CT):
                            ps = spair_tile()
                            for nh in range(QH):
                                for ci in range(CT):
                                    nc.tensor.matmul(
                                        ps[:, nh, :],
                                        wt[:, ci, co * P : (co + 1) * P],
                                        xt[:, ci, nh * NQ : (nh + 1) * NQ],
                                        start=(ci == 0),
                                        stop=(ci == CT - 1),
                                    )
                            nc.scalar.copy(dst[:, co, :], ps[:, 0:2, :])
                    else:
                        off = 0 if mode == "nat_v" else DH
                        for tb in range(TB):
                            ps = spair_tile()
                            for slot, c0, cw in ((0, 0, 512), (1, 512, 256)):
                                for ci in range(CT):
                                    nc.tensor.matmul(
                                        ps[:, slot, :cw],
                                        xt[:, ci, tb * P : (tb + 1) * P],
                                        wt[:, ci, c0 : c0 + cw],
                                        start=(ci == 0),
                                        stop=(ci == CT - 1),
                                    )
                            nc.vector.tensor_copy(
                                vcomb[:, tb, 0:8, off : off + DH],
                                ps[:, 0, :].rearrange("p (h dh) -> p h dh", dh=DH),
                            )
                            nc.vector.tensor_copy(
                                vcomb[:, tb, 8:12, off : off + DH],
                                ps[:, 1, 0:256].rearrange(
                                    "p (h dh) -> p h dh", dh=DH
                                ),
                            )

            def normalize(item):
                # all-SBUF fp16 tensor_tensors (2x DVE mode); partition-shift
                # DMAs fix up the two misaligned halves
                ct, qsl, ub_lo, ub_up, rpb_l, rpb_u = item
                nc.vector.tensor_tensor(
                    axt[0:DH, ct, qsl], ub_lo[0:DH, :], rpb_l[0:DH, :], MULT
                )
                t_il = tmpp.tile([P, NQ], F16, tag="tshift", bufs=4)
                nc.vector.tensor_tensor(
                    t_il[DH:P, :], ub_lo[DH:P, :], rpb_l[DH:P, :], MULT
                )
                nc.sync.dma_start(ait[0:DH, ct, qsl], t_il[DH:P, :])
                t_xu = tmpp.tile([P, NQ], F16, tag="tshift", bufs=4)
                nc.vector.tensor_tensor(
                    t_xu[0:DH, :], ub_up[0:DH, :], rpb_u[0:DH, :], MULT
                )
                nc.sync.dma_start(axt[DH:P, ct, qsl], t_xu[0:DH, :])
                nc.vector.tensor_tensor(
                    ait[DH:P, ct, qsl], ub_up[DH:P, :], rpb_u[DH:P, :], MULT
                )

            def stage2():
                # Cross-group software pipeline: while group g's scores/exps
                # stream, the PE's U/rowsum matmuls consume group g-1's exps
                # (finished ~8 kb earlier). The PE never waits on ACT, so its
                # matmul stream stays back-to-back and the HAM clock-gate
                # ramps to (and stays at) full rate.
                groups = [(ct, qh) for ct in range(CT) for qh in range(QH)]
                stash = []
                prev = None

                def consume(pv, j):
                    # rowsum matmuls use a full 128-wide ones lhsT: keeps FWL
                    # on (M=1 stationary operands disable it and double the
                    # matmul cost on HW) and lands r pre-broadcast on all
                    # 128 partitions, so normalize needs no partition fix-up.
                    u_lo, u_up, r_lo, r_up, es, h_lo, h_up = pv
                    ep = es[j]
                    st, sp = (j == 0), (j == KB - 1)
                    nc.tensor.matmul(
                        u_lo[:], vcomb[:, j, h_lo, :], ep[:, 0, :],
                        start=st, stop=sp,
                    )
                    nc.tensor.matmul(
                        u_up[:], vcomb[:, j, h_up, :], ep[:, 1, :],
                        start=st, stop=sp,
                    )
                    nc.tensor.matmul(
                        r_lo[:], onest[:, 0:P], ep[:, 0, :],
                        start=st, stop=sp,
                    )
                    nc.tensor.matmul(
                        r_up[:], onest[:, 0:P], ep[:, 1, :],
                        start=st, stop=sp,
                    )

                def finish(pv, ct, qsl):
                    # drain U + rowsum recip; normalize deferred
                    u_lo, u_up, r_lo, r_up, es, h_lo, h_up = pv
                    ub_lo = wk.tile([P, NQ], F16, tag="ub", bufs=6)
                    ub_up = wk.tile([P, NQ], F16, tag="ub", bufs=6)
                    nc.vector.tensor_copy(ub_lo[:], u_lo[:])
                    nc.vector.tensor_copy(ub_up[:], u_up[:])
                    rc_l = tmpp.tile([P, NQ], F16, tag="rpb", bufs=6)
                    rc_u = tmpp.tile([P, NQ], F16, tag="rpb", bufs=6)
                    with nc.allow_low_precision(reason="softmax recip fp16"):
                        nc.vector.reciprocal(rc_l[:], r_lo[:])
                        nc.vector.reciprocal(rc_u[:], r_up[:])
                    stash.append((ct, qsl, ub_lo, ub_up, rc_l, rc_u))

                for g, (ct, qh) in enumerate(groups):
                    h_lo, h_up = 2 * ct, 2 * ct + 1
                    qsl = slice(qh * NQ, (qh + 1) * NQ)
                    u_lo = psp.tile([P, NQ], F32, tag="u", bufs=2)
                    u_up = psp.tile([P, NQ], F32, tag="u", bufs=2)
                    r_lo = psp.tile([P, NQ], F32, tag="r", bufs=2)
                    r_up = psp.tile([P, NQ], F32, tag="r", bufs=2)
                    es = []
                    for kb in range(KB):
                        ksl = slice(kb * P, (kb + 1) * P)
                        sp2 = spair_tile()
                        nc.tensor.matmul(
                            sp2[:, 0, :], kbt[0:DH, ct, ksl],
                            qbt[0:DH, ct, qsl], start=True, stop=True,
                        )
                        nc.tensor.matmul(
                            sp2[:, 1, :], kbt[DH:P, ct, ksl],
                            qbt[DH:P, ct, qsl], start=True, stop=True,
                        )
                        ep = wk.tile([P, 2, NQ], F16, tag="e", bufs=16)
                        nc.scalar.activation(
                            ep[:, 0:2, :], sp2[:, 0:2, :], EXP, scale=SCALE
                        )
                        es.append(ep)
                        if prev is not None:
                            consume(prev[0], kb)
                        if kb == 2 and stash:
                            normalize(stash.pop())
                    if prev is not None:
                        finish(prev[0], prev[1], prev[2])
                    prev = ((u_lo, u_up, r_lo, r_up, es, h_lo, h_up), ct, qsl)

                # epilogue: drain the last group's pipeline
                for kb in range(KB):
                    consume(prev[0], kb)
                finish(prev[0], prev[1], prev[2])
                while stash:
                    normalize(stash.pop(0))

            def stage3():
                for dst_dram, src, w_nm, bias_t in (
                    (xo, axt, "wp", bpr),
                    (xio, ait, "wpi", bpir),
                ):
                    wt = wpool.tile([P, CT, C], F16, tag="wt")
                    nc.sync.dma_start(
                        wt[:], d[w_nm].rearrange("(ct p) c -> p ct c", p=P)
                    )
                    for tb in range(TB):
                        ps = spair_tile()
                        for slot, c0, cw in ((0, 0, 512), (1, 512, 256)):
                            nc.tensor.matmul(
                                ps[:, slot, :cw],
                                onest[0:1, 0:P],
                                bias_t[0:1, c0 : c0 + cw],
                                start=True, stop=False,
                            )
                            for ci in range(CT):
                                nc.tensor.matmul(
                                    ps[:, slot, :cw],
                                    src[:, ci, tb * P : (tb + 1) * P],
                                    wt[:, ci, c0 : c0 + cw],
                                    start=False,
                                    stop=(ci == CT - 1),
                                )
                        ot = wk.tile([P, C], F32, tag="ot", bufs=3)
                        nc.scalar.copy(ot[:, 0:512], ps[:, 0, :])
                        nc.scalar.copy(ot[:, 512:768], ps[:, 1, 0:256])
                        nc.sync.dma_start(
                            dst_dram[tb * P : (tb + 1) * P, :], ot[:]
                        )

            def body():
                if "1" in stages:
                    stage1()
                if "2" in stages:
                    stage2()
                if "3" in stages:
                    stage3()

            if loop_n == 1:
                body()
            else:
                with tc.For_i(0, loop_n, 1):
                    body()

    nc.compile()
    return nc


def make_in_maps(q, k, v, v_img, Wq, Wk, Wv, Wvim, Wp, bp, Wpi, bpi, n_cores=8):
    """Host-side prep: per-core transposed fp16 activations + shared fp16 weights."""
    f = np.float32
    h = np.float16
    shared = {
        "wq": np.asarray(Wq, f).T.astype(h),
        "wk": np.asarray(Wk, f).T.astype(h),
        "wv": np.asarray(Wv, f).T.astype(h),
        "wvi": np.asarray(Wvim, f).T.astype(h),
        "wp": np.asarray(Wp, f).T.astype(h),
        "wpi": np.asarray(Wpi, f).T.astype(h),
        "ones": np.ones((P, P), h),
        "bp": np.asarray(bp, f).reshape(1, C).astype(h),
        "bpi": np.asarray(bpi, f).reshape(1, C).astype(h),
    }
    q = np.asarray(q, f)
    k = np.asarray(k, f)
    v = np.asarray(v, f)
    vi = np.asarray(v_img, f)
    in_maps = []
    for b in range(n_cores):
        in_maps.append(
            {
                "xq": np.ascontiguousarray(q[:, b, :].T).astype(h),
                "xk": np.ascontiguousarray(k[:, b, :].T).astype(h),
                "xv": np.ascontiguousarray(v[:, b, :].T).astype(h),
                "xvi": np.ascontiguousarray(vi[:, b, :].T).astype(h),
                **shared,
            }
        )
    return in_maps


# ---------------------------------------------------------------------------
# Harness entry point: full inputs in, full outputs out.
# Shards batch B=8 across the 8 NeuronCores (data parallel), no collectives.
# ---------------------------------------------------------------------------

_NC_CACHE = {}


def _get_module():
    if "nc" not in _NC_CACHE:
        _NC_CACHE["nc"] = build_module(num_devices=8)
    return _NC_CACHE["nc"]


def kernel(q, k, v, v_img, Wq, Wk, Wv, Wvim, Wp, bp, Wpi, bpi):
    from concourse.bass_utils import run_bass_kernel_spmd

    B = np.asarray(q).shape[1]
    nc = _get_module()
    in_maps = make_in_maps(q, k, v, v_img, Wq, Wk, Wv, Wvim, Wp, bp, Wpi, bpi,
                           n_cores=B)
    res = run_bass_kernel_spmd(nc, in_maps, core_ids=list(range(B)), trace=False)
    x = np.stack([res.results[b]["xo"] for b in range(B)])
    x_im = np.stack([res.results[b]["xio"] for b in range(B)])
    return (x, x_im)


# revision 21
# speedup vs baseline: 1.1533x; 1.0768x over previous
"""Dual-stream attention kernel for TRN2 — one batch element per core (v3).

Per-core computation (batch element b):
  qb^T = Wq @ q_b^T          [C, N]   fp16, transposed layout (c on partitions)
  kb^T = Wk @ k_b^T          [C, N]   fp16
  vcomb[tb][tok, h, 0:64]   = (v_b @ Wv^T)    per-head slices   (natural layout)
  vcomb[tb][tok, h, 64:128] = (v_img_b @ Wvim^T)
  per head pair ct (2 heads = one 128-partition q/k tile):
    S^T pair = kh @ qh^T      K=64 matmuls, row-paired (lower/upper head) into
                              one 2-bank PSUM tile
    E = exp(S^T * scale)      ONE ACT instr per kb over both heads [128,1024]
    U = [vh | vih]^T @ E      M=128: rows 0:64 x-stream, 64:128 img-stream
    r = ones^T @ E            M=1 matmuls, col-paired (strip 0 / strip 32),
                              both accumulated in one PSUM bank
    O = U * (1/r)             reciprocal read straight from PSUM; normalize
                              multiplies read U from PSUM (no drain copies);
                              DMA partition-shifts for the two misaligned halves
  x    = merge(O_x)  @ Wp^T  + bp     bias folded in as a K=1 ones-row matmul;
  x_im = merge(O_im) @ Wpi^T + bpi    outputs DMA'd DRAM-ward from PSUM

Engine placement: PE matmuls; ACT exp + stage-1 q/k PSUM drains; DVE vcomb
drains, reciprocals, normalize multiplies. GPSIMD has no PSUM port so it
stays idle. All matmul operands fp16; PSUM accumulation fp32.

build_module(loop_n=N) wraps the body in a hardware For_i loop for wall-clock
timing (amortizes the ~60 ms axon dispatch overhead); timing is
data-independent.
"""

import numpy as np
import concourse.bass as bass
import concourse.tile as tile
from concourse import bacc, mybir

P = 128
NTOK = 1024
C = 768
H = 12
DH = 64
CT = C // P  # 6 c-tiles
TB = NTOK // P  # 8 token blocks
QH = 2  # qt halves
KB = 8  # kt blocks
NQ = 512
SCALE = DH**-0.5
F32 = mybir.dt.float32
F16 = mybir.dt.float16
EXP = mybir.ActivationFunctionType.Exp
MULT = mybir.AluOpType.mult
ADD = mybir.AluOpType.add

XNAMES = ("xq", "xk", "xv", "xvi")
WNAMES = ("wq", "wk", "wv", "wvi", "wp", "wpi")


def build_module(num_devices=8, loop_n=1, stages="123", s2mode="full"):
    nc = bacc.Bacc(
        "TRN2", target_bir_lowering=False, debug=False, num_devices=num_devices
    )
    d = {}
    for nm in XNAMES:
        d[nm] = nc.dram_tensor(nm, [C, NTOK], F16, kind="ExternalInput").ap()
    for nm in WNAMES:
        d[nm] = nc.dram_tensor(nm, [C, C], F16, kind="ExternalInput").ap()
    d["ones"] = nc.dram_tensor("ones", [P, P], F16, kind="ExternalInput").ap()
    d["bp"] = nc.dram_tensor("bp", [1, C], F16, kind="ExternalInput").ap()
    d["bpi"] = nc.dram_tensor("bpi", [1, C], F16, kind="ExternalInput").ap()
    xo = nc.dram_tensor("xo", [NTOK, C], F32, kind="ExternalOutput").ap()
    xio = nc.dram_tensor("xio", [NTOK, C], F32, kind="ExternalOutput").ap()

    with tile.TileContext(nc) as tc:
        with (
            tc.tile_pool(name="persist", bufs=1) as pp,
            tc.tile_pool(name="wstage", bufs=2) as wpool,
            tc.tile_pool(name="xstage", bufs=2) as xpool,
            tc.tile_pool(name="wk", bufs=8) as wk,
            tc.tile_pool(name="nrm", bufs=8) as nrm,
            tc.tile_pool(name="tmp", bufs=8) as tmpp,
            tc.tile_pool(name="ps", bufs=1, space="PSUM") as psp,
        ):
            qbt = pp.tile([P, CT, NTOK], F16, tag="qbt")
            kbt = pp.tile([P, CT, NTOK], F16, tag="kbt")
            # [v | vi] per head: lhsT for the combined AV matmul
            vcomb = pp.tile([P, TB, H, P], F16, tag="vcomb")
            axt = pp.tile([P, CT, NTOK], F16, tag="axt")
            ait = pp.tile([P, CT, NTOK], F16, tag="ait")
            onest = pp.tile([P, P], F16, tag="onest")
            bpr = pp.tile([1, C], F16, tag="bpr")
            bpir = pp.tile([1, C], F16, tag="bpir")
            edum = (
                pp.tile([P, 2, NQ], F16, tag="edum")
                if s2mode in ("dumep", "noub") else None
            )

            # PSUM budget (8 banks): spair 2x2 + u 2 + r 1 + rp 1
            def spair_tile():
                return psp.tile(
                    [P, 2, NQ], F32, tag="spair", bufs=2, name="spair"
                )

            def stage1():
                if s2mode == "dumep":
                    nc.vector.memset(edum[:], 0.01)
                nc.sync.dma_start(bpr[:], d["bp"])
                nc.sync.dma_start(bpir[:], d["bpi"])
                nc.sync.dma_start(onest[:], d["ones"])

                for src, wsrc, mode in (
                    ("xv", "wv", "nat_v"),
                    ("xvi", "wvi", "nat_vi"),
                    ("xq", "wq", "tr_q"),
                    ("xk", "wk", "tr_k"),
                ):
                    xt = xpool.tile([P, CT, NTOK], F16, tag="xt")
                    nc.sync.dma_start(
                        xt[:], d[src].rearrange("(ct p) n -> p ct n", p=P)
                    )
                    wt = wpool.tile([P, CT, C], F16, tag="wt")
                    nc.sync.dma_start(
                        wt[:], d[wsrc].rearrange("(ct p) c -> p ct c", p=P)
                    )
                    if mode.startswith("tr"):
                        dst = qbt if mode == "tr_q" else kbt
                        for co in range(CT):
                            ps = spair_tile()
                            for nh in range(QH):
                                for ci in range(CT):
                                    nc.tensor.matmul(
                                        ps[:, nh, :],
                                        wt[:, ci, co * P : (co + 1) * P],
                                        xt[:, ci, nh * NQ : (nh + 1) * NQ],
                                        start=(ci == 0),
                                        stop=(ci == CT - 1),
                                    )
                            nc.scalar.copy(dst[:, co, :], ps[:, 0:2, :])
                    else:
                        off = 0 if mode == "nat_v" else DH
                        for tb in range(TB):
                            ps = spair_tile()
                            for slot, c0, cw in ((0, 0, 512), (1, 512, 256)):
                                for ci in range(CT):
                                    nc.tensor.matmul(
                                        ps[:, slot, :cw],
                                        xt[:, ci, tb * P : (tb + 1) * P],
                                        wt[:, ci, c0 : c0 + cw],
                                        start=(ci == 0),
                                        stop=(ci == CT - 1),
                                    )
                            nc.vector.tensor_copy(
                                vcomb[:, tb, 0:8, off : off + DH],
                                ps[:, 0, :].rearrange("p (h dh) -> p h dh", dh=DH),
                            )
                            nc.vector.tensor_copy(
                                vcomb[:, tb, 8:12, off : off + DH],
                                ps[:, 1, 0:256].rearrange(
                                    "p (h dh) -> p h dh", dh=DH
                                ),
                            )

            def normalize(item):
                # all-SBUF fp16 tensor_tensors (2x DVE mode); partition-shift
                # DMAs fix up the two misaligned halves
                if item is None or s2mode == "nonorm":
                    return
                qh_, ct, qsl, ub_lo, ub_up, rpb_l, rpb_u = item
                nc.vector.tensor_tensor(
                    axt[0:DH, ct, qsl], ub_lo[0:DH, :], rpb_l[0:DH, :], MULT
                )
                t_il = tmpp.tile([P, NQ], F16, tag="tshift", bufs=2)
                nc.vector.tensor_tensor(
                    t_il[DH:P, :], ub_lo[DH:P, :], rpb_l[DH:P, :], MULT
                )
                if s2mode != "noshift":
                    nc.sync.dma_start(ait[0:DH, ct, qsl], t_il[DH:P, :])
                t_xu = tmpp.tile([P, NQ], F16, tag="tshift", bufs=2)
                nc.vector.tensor_tensor(
                    t_xu[0:DH, :], ub_up[0:DH, :], rpb_u[0:DH, :], MULT
                )
                if s2mode != "noshift":
                    nc.sync.dma_start(axt[DH:P, ct, qsl], t_xu[0:DH, :])
                nc.vector.tensor_tensor(
                    ait[DH:P, ct, qsl], ub_up[DH:P, :], rpb_u[DH:P, :], MULT
                )

            def stage2():
                # Cross-group software pipeline: while group g's scores/exps
                # stream, the PE's U/rowsum matmuls consume group g-1's exps
                # (finished ~8 kb earlier). The PE never waits on ACT, so its
                # matmul stream stays back-to-back and the HAM clock-gate
                # ramps to (and stays at) full rate.
                groups = [(ct, qh) for ct in range(CT) for qh in range(QH)]
                stash = []
                prev = None

                def consume(pv, j):
                    # rowsum matmuls use a full 128-wide ones lhsT: keeps FWL
                    # on (M=1 stationary operands disable it and double the
                    # matmul cost on HW) and lands r pre-broadcast on all
                    # 128 partitions, so normalize needs no partition fix-up.
                    u_lo, u_up, r_lo, r_up, es, h_lo, h_up = pv
                    ep = edum if s2mode == "dumep" else es[j]
                    st, sp = (j == 0), (j == KB - 1)
                    nc.tensor.matmul(
                        u_lo[:], vcomb[:, j, h_lo, :], ep[:, 0, :],
                        start=st, stop=sp,
                    )
                    nc.tensor.matmul(
                        u_up[:], vcomb[:, j, h_up, :], ep[:, 1, :],
                        start=st, stop=sp,
                    )
                    nc.tensor.matmul(
                        r_lo[:], onest[:, 0:P], ep[:, 0, :],
                        start=st, stop=sp,
                    )
                    nc.tensor.matmul(
                        r_up[:], onest[:, 0:P], ep[:, 1, :],
                        start=st, stop=sp,
                    )

                def finish(pv, ct, qsl):
                    # drain U + rowsum recip; normalize deferred
                    u_lo, u_up, r_lo, r_up, es, h_lo, h_up = pv
                    if s2mode == "noub":
                        nc.vector.tensor_copy(edum[:, 0, :], u_lo[:])
                        nc.vector.tensor_copy(edum[:, 1, :], u_up[:])
                        nc.vector.tensor_copy(edum[0:1, 0, :], r_lo[0:1, :])
                        nc.vector.tensor_copy(edum[0:1, 1, :], r_up[0:1, :])
                        return
                    ub_lo = wk.tile([P, NQ], F16, tag="ub", bufs=24)
                    ub_up = wk.tile([P, NQ], F16, tag="ub", bufs=24)
                    nc.vector.tensor_copy(ub_lo[:], u_lo[:])
                    nc.vector.tensor_copy(ub_up[:], u_up[:])
                    rc_l = tmpp.tile([P, NQ], F16, tag="rpb", bufs=24)
                    rc_u = tmpp.tile([P, NQ], F16, tag="rpb", bufs=24)
                    if s2mode == "norecip":
                        nc.vector.memset(rc_l[:], 1.0)
                        nc.vector.memset(rc_u[:], 1.0)
                    else:
                        with nc.allow_low_precision(reason="softmax recip fp16"):
                            nc.vector.reciprocal(rc_l[:], r_lo[:])
                            nc.vector.reciprocal(rc_u[:], r_up[:])
                    stash.append((qsl.start // NQ, ct, qsl, ub_lo, ub_up, rc_l, rc_u))

                for g, (ct, qh) in enumerate(groups):
                    h_lo, h_up = 2 * ct, 2 * ct + 1
                    qsl = slice(qh * NQ, (qh + 1) * NQ)
                    u_lo = psp.tile([P, NQ], F32, tag="u", bufs=2)
                    u_up = psp.tile([P, NQ], F32, tag="u", bufs=2)
                    r_lo = psp.tile([P, NQ], F32, tag="r", bufs=2)
                    r_up = psp.tile([P, NQ], F32, tag="r", bufs=2)
                    es = []
                    for kb in range(KB):
                        ksl = slice(kb * P, (kb + 1) * P)
                        sp2 = spair_tile()
                        nc.tensor.matmul(
                            sp2[:, 0, :], kbt[0:DH, ct, ksl],
                            qbt[0:DH, ct, qsl], start=True, stop=True,
                        )
                        nc.tensor.matmul(
                            sp2[:, 1, :], kbt[DH:P, ct, ksl],
                            qbt[DH:P, ct, qsl], start=True, stop=True,
                        )
                        ep = wk.tile([P, 2, NQ], F16, tag="e", bufs=15)
                        if s2mode == "exp1":
                            nc.scalar.activation(
                                ep[:, 0:2, :], sp2[:, 0:2, :], EXP, scale=SCALE
                            )
                        else:
                            # single-bank ACT PSUM reads measure faster than one
                            # fused 2-bank read
                            nc.scalar.activation(
                                ep[:, 0, :], sp2[:, 0, :], EXP, scale=SCALE
                            )
                            nc.scalar.activation(
                                ep[:, 1, :], sp2[:, 1, :], EXP, scale=SCALE
                            )
                        es.append(ep)
                        if prev is not None:
                            consume(prev[0], kb)
                    if prev is not None:
                        finish(prev[0], prev[1], prev[2])
                    prev = ((u_lo, u_up, r_lo, r_up, es, h_lo, h_up), ct, qsl)

                # epilogue: drain the last group's pipeline, then run the
                # whole normalize pass qh-major (its DVE work poisons the
                # in-loop PE/ACT stream; here it overlaps stage3 instead)
                for kb in range(KB):
                    consume(prev[0], kb)
                finish(prev[0], prev[1], prev[2])
                for item in sorted(stash, key=lambda it: (it[0], it[1])):
                    normalize(item)
                stash.clear()

            def stage3():
                for dst_dram, src_t, w_nm, bias_t in (
                    (xo, axt, "wp", bpr),
                    (xio, ait, "wpi", bpir),
                ):
                    wt = wpool.tile([P, CT, C], F16, tag="wt")
                    nc.sync.dma_start(
                        wt[:], d[w_nm].rearrange("(ct p) c -> p ct c", p=P)
                    )
                    for tb in range(TB):
                        for ci_t, (c0, cw) in enumerate(((0, 512), (512, 256))):
                            tag = ("u", "r")[(2 * tb + ci_t) % 4 // 2]
                            ps = psp.tile([P, NQ], F32, tag=tag, bufs=2, name="s3ps")
                            nc.tensor.matmul(
                                ps[:, :cw],
                                onest[0:1, 0:P],
                                bias_t[0:1, c0 : c0 + cw],
                                start=True, stop=False,
                            )
                            for ci in range(CT):
                                nc.tensor.matmul(
                                    ps[:, :cw],
                                    src_t[:, ci, tb * P : (tb + 1) * P],
                                    wt[:, ci, c0 : c0 + cw],
                                    start=False,
                                    stop=(ci == CT - 1),
                                )
                            ot = wk.tile([P, NQ], F32, tag="ot", bufs=4)
                            nc.vector.tensor_copy(ot[:, :cw], ps[:, :cw])
                            nc.sync.dma_start(
                                dst_dram[tb * P : (tb + 1) * P, c0 : c0 + cw],
                                ot[:, :cw],
                            )

            def body():
                if "1" in stages:
                    stage1()
                if "2" in stages:
                    stage2()
                if "3" in stages:
                    stage3()

            if loop_n == 1:
                body()
            else:
                with tc.For_i(0, loop_n, 1):
                    body()

    nc.compile()
    return nc


def make_in_maps(q, k, v, v_img, Wq, Wk, Wv, Wvim, Wp, bp, Wpi, bpi, n_cores=8):
    """Host-side prep: per-core transposed fp16 activations + shared fp16 weights."""
    f = np.float32
    h = np.float16
    shared = {
        "wq": np.asarray(Wq, f).T.astype(h),
        "wk": np.asarray(Wk, f).T.astype(h),
        "wv": np.asarray(Wv, f).T.astype(h),
        "wvi": np.asarray(Wvim, f).T.astype(h),
        "wp": np.asarray(Wp, f).T.astype(h),
        "wpi": np.asarray(Wpi, f).T.astype(h),
        "ones": np.ones((P, P), h),
        "bp": np.asarray(bp, f).reshape(1, C).astype(h),
        "bpi": np.asarray(bpi, f).reshape(1, C).astype(h),
    }
    q = np.asarray(q, f)
    k = np.asarray(k, f)
    v = np.asarray(v, f)
    vi = np.asarray(v_img, f)
    in_maps = []
    for b in range(n_cores):
        in_maps.append(
            {
                "xq": np.ascontiguousarray(q[:, b, :].T).astype(h),
                "xk": np.ascontiguousarray(k[:, b, :].T).astype(h),
                "xv": np.ascontiguousarray(v[:, b, :].T).astype(h),
                "xvi": np.ascontiguousarray(vi[:, b, :].T).astype(h),
                **shared,
            }
        )
    return in_maps


# ---------------------------------------------------------------------------
# Harness entry point: full inputs in, full outputs out.
# Shards batch B=8 across the 8 NeuronCores (data parallel), no collectives.
# ---------------------------------------------------------------------------

_NC_CACHE = {}


def _get_module():
    if "nc" not in _NC_CACHE:
        _NC_CACHE["nc"] = build_module(num_devices=8)
    return _NC_CACHE["nc"]


def kernel(q, k, v, v_img, Wq, Wk, Wv, Wvim, Wp, bp, Wpi, bpi):
    from concourse.bass_utils import run_bass_kernel_spmd

    B = np.asarray(q).shape[1]
    nc = _get_module()
    in_maps = make_in_maps(q, k, v, v_img, Wq, Wk, Wv, Wvim, Wp, bp, Wpi, bpi,
                           n_cores=B)
    res = run_bass_kernel_spmd(nc, in_maps, core_ids=list(range(B)), trace=False)
    x = np.stack([res.results[b]["xo"] for b in range(B)])
    x_im = np.stack([res.results[b]["xio"] for b in range(B)])
    return (x, x_im)
